# revision 22
# baseline (speedup 1.0000x reference)
"""KPCNN (kernel-predicting CNN) Trainium2 Bass kernel.

Strategy (hardcoded for B=32768, 8 cores, pure data parallel, 4096 samples/core):
 - All convs on 5x5 spatial are reformulated as dense matmuls over flattened
   (pixel, channel) feature vectors, row-banded by output image row so each
   125-wide output chunk contracts only the 2-3 input row chunks in its 3x3
   band (13 [125x125] blocks per 25->25 layer instead of 25).
 - Activations are feature-major [feat, batch] in SBUF, fp32r (TF32-like,
   full PE rate at N>=256), PSUM accumulate fp32.
 - Entry: PE-transpose of naturally-DMA'd [128 samples, 200 feat] tiles.
 - Tail (softmax over 6 predicted weights + per-pixel color mix) runs
   sample-major after PE-transposing back, on DVE/ACT.
 - Host<->device I/O is fp16 (wall-clock is dominated by the axon tunnel,
   ~84MB/s + ~60ms/op fixed), weights + compiled executable are cached on
   device across calls, and the donated output buffer is produced on-device
   by a tiny cached jit instead of shipping zeros over the tunnel.
Weight densification happens on host (weights are tiny).
"""
import sys
sys.path.insert(0, '/opt/trn_rl_repo')
import hashlib
import numpy as np

B_TOTAL = 32768
N_CORES = 8
N_PER_CORE = B_TOTAL // N_CORES   # 4096
NT = 512                          # samples per supertile
NUM_MID = 6
N_CHUNKS = 1                      # execs per call (1: in-order cmd queue makes
                                  # any exec between puts stall the H2D stream)
N_XPARTS = 1                      # x dram tensors per core (1: upload is per
                                  # device instead, in device-major order)
NPC = N_PER_CORE // N_CHUNKS

_CACHE = {}


def _band(y):
    return [yi for yi in (y - 1, y, y + 1) if 0 <= yi <= 4]


def _densify_mid(w):
    """w [25,25,3,3] OIHW -> [13,125,125] blocks (lhsT: [q_in, q_out])."""
    blocks = np.zeros((13, 125, 125), np.float32)
    bi = 0
    for y_out in range(5):
        for y_in in _band(y_out):
            dy = y_in - y_out
            for x_in in range(5):
                for x_out in range(5):
                    dx = x_in - x_out
                    if abs(dx) <= 1:
                        blocks[bi, x_in*25:(x_in+1)*25, x_out*25:(x_out+1)*25] = \
                            w[:, :, dy+1, dx+1].T
            bi += 1
    return blocks


def _densify_conv0(w):
    """w [25,8,3,3] -> [200,625]: row c_in*25+y_in*5+x_in, col y_out*125+x_out*25+c_out."""
    W = np.zeros((200, 625), np.float32)
    for y_in in range(5):
        for x_in in range(5):
            for y_out in range(5):
                dy = y_in - y_out
                if abs(dy) > 1:
                    continue
                for x_out in range(5):
                    dx = x_in - x_out
                    if abs(dx) > 1:
                        continue
                    for c_in in range(8):
                        W[c_in*25 + y_in*5 + x_in,
                          y_out*125 + x_out*25:y_out*125 + x_out*25 + 25] = \
                            w[:, c_in, dy+1, dx+1]
    return W


def _densify_last(w):
    """w [6,25,3,3] -> [625,150]: row y_in*125+x_in*25+c_in, col wi*25+y_out*5+x_out."""
    W = np.zeros((625, 150), np.float32)
    for y_in in range(5):
        for x_in in range(5):
            for y_out in range(5):
                dy = y_in - y_out
                if abs(dy) > 1:
                    continue
                for x_out in range(5):
                    dx = x_in - x_out
                    if abs(dx) > 1:
                        continue
                    for c_in in range(25):
                        for wi in range(6):
                            W[y_in*125 + x_in*25 + c_in, wi*25 + y_out*5 + x_out] = \
                                w[wi, c_in, dy+1, dx+1]
    return W


def _build(npc):
    import concourse.bass as bass
    from concourse import bacc
    import concourse.tile as tile
    import concourse.mybir as mybir

    dt = mybir.dt
    AF = mybir.ActivationFunctionType
    ALU = mybir.AluOpType

    nc = bacc.Bacc("TRN2", target_bir_lowering=False, debug=False)
    n_st = npc // NT

    f32, f32r, f16 = dt.float32, dt.float32r, dt.float16
    npp = npc // N_XPARTS            # rows per x part
    x_ds = [nc.dram_tensor(f"x{p}", [npp, 200], f16, kind="ExternalInput").ap()
            for p in range(N_XPARTS)]
    y_d = nc.dram_tensor("y", [npc, 75], f16, kind="ExternalOutput").ap()
    w0a_d = nc.dram_tensor("w0a", [128, 625], f32, kind="ExternalInput").ap()
    w0b_d = nc.dram_tensor("w0b", [72, 625], f32, kind="ExternalInput").ap()
    wm_d = nc.dram_tensor("wm", [125, NUM_MID, 13, 125], f32, kind="ExternalInput").ap()
    wl_d = nc.dram_tensor("wl", [125, 5, 150], f32, kind="ExternalInput").ap()
    wp_d = nc.dram_tensor("wp", [75, 18], f32, kind="ExternalInput").ap()
    id_d = nc.dram_tensor("ident", [128, 128], f32, kind="ExternalInput").ap()
    id16_d = nc.dram_tensor("ident16", [128, 128], f16, kind="ExternalInput").ap()
    b0_d = nc.dram_tensor("b0q", [125, 1], f32, kind="ExternalInput").ap()
    bm_d = nc.dram_tensor("bmq", [125, NUM_MID], f32, kind="ExternalInput").ap()
    bl_d = nc.dram_tensor("blq", [75, 2], f32, kind="ExternalInput").ap()
    bp_d = nc.dram_tensor("bpq", [18, 1], f32, kind="ExternalInput").ap()

    with tile.TileContext(nc) as tc:
        with tc.tile_pool(name="wpool", bufs=1) as wpool, \
             tc.tile_pool(name="apool", bufs=3) as apool, \
             tc.tile_pool(name="npool", bufs=6) as npool, \
             tc.tile_pool(name="tpool", bufs=6) as tpool, \
             tc.tile_pool(name="pspool", bufs=8, space="PSUM") as pspool:

            w0a = wpool.tile([128, 625], f32r)
            w0b = wpool.tile([72, 625], f32r)
            wm = wpool.tile([125, NUM_MID, 13, 125], f32r)
            wl = wpool.tile([125, 5, 150], f32r)
            wp = wpool.tile([75, 18], f32r)
            ident = wpool.tile([128, 128], f32r)
            ident16 = wpool.tile([128, 128], f16)
            b0q = wpool.tile([125, 1], f32)
            bmq = wpool.tile([125, NUM_MID], f32)
            blq = wpool.tile([75, 2], f32)
            bpq = wpool.tile([18, 1], f32)
            nc.sync.dma_start(out=w0a, in_=w0a_d.bitcast(f32r))
            nc.sync.dma_start(out=w0b, in_=w0b_d.bitcast(f32r))
            nc.sync.dma_start(out=wm, in_=wm_d.bitcast(f32r))
            nc.sync.dma_start(out=wl, in_=wl_d.bitcast(f32r))
            nc.sync.dma_start(out=wp, in_=wp_d.bitcast(f32r))
            nc.sync.dma_start(out=ident, in_=id_d.bitcast(f32r))
            nc.sync.dma_start(out=ident16, in_=id16_d)
            nc.sync.dma_start(out=b0q, in_=b0_d)
            nc.sync.dma_start(out=bmq, in_=bm_d)
            nc.sync.dma_start(out=blq, in_=bl_d)
            nc.sync.dma_start(out=bpq, in_=bp_d)

            for s in range(n_st):
                base = s * NT
                # --- entry: DMA natural fp16 tiles, PE-transpose to feature-major
                xA = apool.tile([128, NT], f32r)
                xB = apool.tile([72, NT], f32r)
                for g in range(4):
                    nat = npool.tile([128, 200], f16, tag="nat")
                    r0 = base + g*128
                    nc.sync.dma_start(
                        out=nat, in_=x_ds[r0 // npp][r0 % npp:r0 % npp + 128, :])
                    psA = pspool.tile([128, 128], f16, tag="ps")
                    nc.tensor.transpose(psA, nat[:, 0:128], ident16)
                    nc.vector.tensor_copy(xA[:, g*128:(g+1)*128], psA)
                    psB = pspool.tile([72, 128], f16, tag="ps")
                    nc.tensor.transpose(psB, nat[:, 128:200], ident16)
                    nc.vector.tensor_copy(xB[:, g*128:(g+1)*128], psB)

                # --- conv0 (dense 200->625)
                h = apool.tile([125, 5, NT], f32r, tag="h")
                for y in range(5):
                    ps = pspool.tile([125, NT], f32, tag="ps")
                    nc.tensor.matmul(ps, w0a[:, y*125:(y+1)*125], xA,
                                     start=True, stop=False)
                    nc.tensor.matmul(ps, w0b[:, y*125:(y+1)*125], xB,
                                     start=False, stop=True)
                    if y >= 3:  # balance eviction load ACT vs DVE
                        nc.vector.tensor_scalar(h[:, y, :], ps, b0q, 0.0,
                                                op0=ALU.add, op1=ALU.max)
                    else:
                        nc.scalar.activation(h[:, y, :], ps, AF.Relu, bias=b0q)

                # --- 6 mid layers (row-banded 625->625)
                for l in range(NUM_MID):
                    hn = apool.tile([125, 5, NT], f32r, tag="h")
                    for y in range(5):
                        bnd = _band(y)
                        bi = sum(len(_band(yy)) for yy in range(y))
                        ps = pspool.tile([125, NT], f32, tag="ps")
                        for j, y_in in enumerate(bnd):
                            nc.tensor.matmul(ps, wm[:, l, bi+j, :], h[:, y_in, :],
                                             start=(j == 0), stop=(j == len(bnd)-1))
                        if y >= 3:
                            nc.vector.tensor_scalar(hn[:, y, :], ps,
                                                    bmq[:, l:l+1], 0.0,
                                                    op0=ALU.add, op1=ALU.max)
                        else:
                            nc.scalar.activation(hn[:, y, :], ps, AF.Relu,
                                                 bias=bmq[:, l:l+1])
                    h = hn

                # --- last layer (625->150, logits, w-major cols)
                hl = apool.tile([75, 2, NT], f32r)
                for m in range(2):
                    ps = pspool.tile([75, NT], f32, tag="ps")
                    for k in range(5):
                        nc.tensor.matmul(ps, wl[:, k, m*75:(m+1)*75], h[:, k, :],
                                         start=(k == 0), stop=(k == 4))
                    nc.scalar.activation(hl[:, m, :], ps, AF.Identity,
                                         bias=blq[:, m:m+1])

                # --- post conv (colors: 75->18)
                colors = apool.tile([18, NT], f32r)
                psc = pspool.tile([18, NT], f32, tag="ps")
                nc.tensor.matmul(psc, wp, xA[0:75, :], start=True, stop=True)
                nc.scalar.activation(colors, psc, AF.Identity, bias=bpq)

                # --- tail: per 128-group, sample-major softmax + color mix
                for g in range(4):
                    gs = slice(g*128, (g+1)*128)
                    # fp32r matmul ISA restriction: innermost free n_step must
                    # be even on moving operand and dst -> pad 75 to 76.
                    tE0 = pspool.tile([128, 76], f32r, tag="ps")
                    nc.tensor.transpose(tE0, hl[:, 0, gs], ident[0:75, 0:76])
                    tE1 = pspool.tile([128, 76], f32r, tag="ps")
                    nc.tensor.transpose(tE1, hl[:, 1, gs], ident[0:75, 0:76])
                    E = tpool.tile([128, 150], f32, tag="E")
                    nc.scalar.activation(E[:, 0:75], tE0[:, 0:75], AF.Exp)
                    nc.scalar.activation(E[:, 75:150], tE1[:, 0:75], AF.Exp)
                    tC = pspool.tile([128, 18], f32r, tag="ps")
                    nc.tensor.transpose(tC, colors[:, gs], ident[0:18, 0:18])
                    colT = tpool.tile([128, 18], f32, tag="colT")
                    nc.scalar.activation(colT, tC, AF.Copy)

                    S = tpool.tile([128, 25], f32, tag="S")
                    nc.vector.tensor_reduce(
                        out=S, in_=E.rearrange("p (w q) -> p q w", w=6),
                        axis=mybir.AxisListType.X, op=ALU.add)
                    R = tpool.tile([128, 25], f32, tag="R")
                    nc.vector.reciprocal(R, S)

                    U = tpool.tile([128, 3, 25], f32, tag="U")
                    for c in range(3):
                        nc.vector.tensor_scalar_mul(
                            U[:, c, :], E[:, 0:25], colT[:, c*6:c*6+1])
                        for w in range(1, 6):
                            nc.vector.scalar_tensor_tensor(
                                out=U[:, c, :], in0=E[:, w*25:(w+1)*25],
                                scalar=colT[:, c*6+w:c*6+w+1], in1=U[:, c, :],
                                op0=ALU.mult, op1=ALU.add)
                    F = tpool.tile([128, 3, 25], f16, tag="F")
                    nc.vector.tensor_tensor(
                        out=F, in0=U,
                        in1=R.unsqueeze(1).broadcast_to([128, 3, 25]),
                        op=ALU.mult)
                    nc.sync.dma_start(
                        out=y_d[base+g*128:base+(g+1)*128, :],
                        in_=F.rearrange("p a b -> p (a b)"))

    nc.compile()
    return nc


def _prep_weights(w0, b0, wmid, bmid, wlast, blast, wpost, bpost):
    W0 = _densify_conv0(np.asarray(w0, np.float32))
    wm = np.zeros((125, NUM_MID, 13, 125), np.float32)
    for l in range(NUM_MID):
        blocks = _densify_mid(np.asarray(wmid[l], np.float32))
        for bi in range(13):
            wm[:, l, bi, :] = blocks[bi]
    Wl = _densify_last(np.asarray(wlast, np.float32))
    wl = np.ascontiguousarray(
        np.transpose(Wl.reshape(5, 125, 150), (1, 0, 2)))
    wp = np.ascontiguousarray(
        np.asarray(wpost, np.float32).reshape(18, 75).T)
    b0q = np.tile(np.asarray(b0, np.float32), 5)[:, None]
    bmq = np.stack([np.tile(np.asarray(bmid[l], np.float32), 5)
                    for l in range(NUM_MID)], axis=1)
    blq = np.asarray(blast, np.float32).repeat(25).reshape(2, 75).T
    bpq = np.asarray(bpost, np.float32)[:, None]
    return {
        "w0a": np.ascontiguousarray(W0[0:128]),
        "w0b": np.ascontiguousarray(W0[128:200]),
        "wm": wm, "wl": wl, "wp": wp,
        "ident": np.eye(128, dtype=np.float32),
        "ident16": np.eye(128, dtype=np.float16),
        "b0q": np.ascontiguousarray(b0q), "bmq": np.ascontiguousarray(bmq),
        "blq": np.ascontiguousarray(blq), "bpq": bpq,
    }


def _get_exec(npc=NPC):
    """Build + compile the Bass program once per per-core batch size; cache
    a reusable jitted executable (the axon path re-traces jax.jit per call
    otherwise)."""
    if ("exec", npc) in _CACHE:
        return _CACHE[("exec", npc)]
    import jax
    import jax.numpy as jnp
    from jax.sharding import Mesh, PartitionSpec, NamedSharding
    from jax.experimental.shard_map import shard_map
    from concourse import mybir
    from concourse.bass2jax import (
        _bass_exec_p, install_neuronx_cc_hook, partition_id_tensor)

    install_neuronx_cc_hook()
    nc = _build(npc)

    partition_name = (nc.partition_id_tensor.name
                      if nc.partition_id_tensor else None)
    in_names, out_names, out_avals = [], [], []
    for alloc in nc.m.functions[0].allocations:
        if not isinstance(alloc, mybir.MemoryLocationSet):
            continue
        name = alloc.memorylocations[0].name
        if alloc.kind == "ExternalInput":
            if name != partition_name:
                in_names.append(name)
        elif alloc.kind == "ExternalOutput":
            out_names.append(name)
            out_avals.append(jax.core.ShapedArray(
                tuple(alloc.tensor_shape), mybir.dt.np(alloc.dtype)))
    n_params = len(in_names)
    in_names_full = in_names + out_names + (
        [partition_name] if partition_name else [])

    def _body(*args):
        operands = list(args)
        if partition_name is not None:
            operands.append(partition_id_tensor())
        outs = _bass_exec_p.bind(
            *operands, out_avals=tuple(out_avals),
            in_names=tuple(in_names_full), out_names=tuple(out_names),
            lowering_input_output_aliases=(), sim_require_finite=True,
            sim_require_nnan=True, nc=nc)
        return tuple(outs)

    devices = jax.devices()[:N_CORES]
    mesh = Mesh(np.asarray(devices), ("core",))
    S8 = NamedSharding(mesh, PartitionSpec("core"))
    n_outs = len(out_names)
    sharded = jax.jit(
        shard_map(_body, mesh=mesh,
                  in_specs=(PartitionSpec("core"),) * (n_params + n_outs),
                  out_specs=(PartitionSpec("core"),) * n_outs,
                  check_rep=False),
        donate_argnums=tuple(range(n_params, n_params + n_outs)),
        keep_unused=True)
    zeros_jit = jax.jit(
        lambda: jnp.zeros((N_CORES * npc, 75), jnp.float16),
        out_shardings=S8)

    ex = {"nc": nc, "sharded": sharded, "zeros_jit": zeros_jit,
          "in_names": in_names, "sharding": S8, "jax": jax,
          "ybufs": [], "npc": npc}
    _CACHE[("exec", npc)] = ex
    return ex


FETCH_THREADS = 8     # <=1 : serial np.asarray on the global sharded array
_POOL = None


def _pool():
    global _POOL
    if _POOL is None:
        from concurrent.futures import ThreadPoolExecutor
        _POOL = ThreadPoolExecutor(max_workers=8)
    return _POOL


def _fetch_into(y, out_view):
    """Device sharded [R,75] f16 -> out_view [N_CORES, R//N_CORES, 75] f32."""
    if FETCH_THREADS <= 1:
        out_view[...] = np.asarray(y).reshape(out_view.shape)
        return
    shards = y.addressable_shards
    def fx(i):
        s = shards[i]
        out_view[i] = np.asarray(s.data).reshape(out_view.shape[1:])
    list(_pool().map(fx, range(len(shards))))


def kernel(input, w0, b0, wmid, bmid, wlast, blast, wpost, bpost, _trace=False):
    npc = N_PER_CORE // N_CHUNKS
    ex = _get_exec(npc)
    jax = ex["jax"]

    # cache densified+device-resident weights keyed by raw weight bytes
    hsh = hashlib.blake2b(digest_size=16)
    for a in (w0, b0, wmid, bmid, wlast, blast, wpost, bpost):
        hsh.update(np.ascontiguousarray(a).tobytes())
    wkey = hsh.hexdigest()
    if _CACHE.get("wkey") != wkey:
        wmap = _prep_weights(w0, b0, wmid, bmid, wlast, blast, wpost, bpost)
        dev_w = {}
        for name, arr in wmap.items():
            rep = np.concatenate([arr] * N_CORES, axis=0)
            dev_w[name] = jax.device_put(rep, ex["sharding"])
        _CACHE["dev_w"] = dev_w
        _CACHE["wkey"] = wkey

    # Issue order matters on the axon tunnel (in-order command queue, async
    # pulls): ALL uploads first, then all execs, then fetches. Each exec is
    # gated only on its own chunk's transfer, so chunk k's exec + y fetch
    # (slow D2H direction, ~35MB/s) overlap the upload of chunks k+1.. An
    # exec issued BETWEEN puts would stall the upload stream (~75ms each).
    # The donated output buffers are the previous call's device-resident
    # outputs (the kernel writes every element, so contents don't matter)
    # -- no per-call zero-fill exec.
    xv = np.asarray(input).reshape(N_CORES, N_CHUNKS, npc, 200)
    ybufs = ex["ybufs"]
    while len(ybufs) < N_CHUNKS:
        ybufs.append(ex["zeros_jit"]())
    warg = [_CACHE["dev_w"].get(n) for n in ex["in_names"]]
    xi = ex["in_names"].index("x0")
    xds = []
    for k in range(N_CHUNKS):
        x16 = np.empty((N_CORES, npc, 200), np.float16)
        x16[...] = xv[:, k]
        xds.append(jax.device_put(
            x16.reshape(N_CORES * npc, 200), ex["sharding"]))
    ys = []
    for k in range(N_CHUNKS):
        warg[xi] = xds[k]
        (y,) = ex["sharded"](*warg, ybufs[k])
        ys.append(y)
    out = np.empty((N_CORES, N_CHUNKS, npc, 75), np.float32)
    for k in range(N_CHUNKS):
        _fetch_into(ys[k], out[:, k])
    ex["ybufs"] = ys   # donate these next call
    return out.reshape(B_TOTAL, 75).reshape(B_TOTAL, 3, 5, 5)


# revision 26
# speedup vs baseline: 1.0345x; 1.0345x over previous
"""KPCNN (kernel-predicting CNN) Trainium2 Bass kernel.

Strategy (hardcoded for B=32768, 8 cores, pure data parallel, 4096 samples/core):
 - All convs on 5x5 spatial are reformulated as dense matmuls over flattened
   (pixel, channel) feature vectors, row-banded by output image row so each
   125-wide output chunk contracts only the 2-3 input row chunks in its 3x3
   band (13 [125x125] blocks per 25->25 layer instead of 25).
 - Activations are feature-major [feat, batch] in SBUF, fp32r (TF32-like,
   full PE rate at N>=256), PSUM accumulate fp32.
 - Entry: PE-transpose of naturally-DMA'd [128 samples, 200 feat] tiles.
 - Tail (softmax over 6 predicted weights + per-pixel color mix) runs
   sample-major after PE-transposing back, on DVE/ACT.
 - Host<->device I/O is fp16 (wall-clock is dominated by the axon tunnel,
   ~84MB/s + ~60ms/op fixed), weights + compiled executable are cached on
   device across calls, and the donated output buffer is produced on-device
   by a tiny cached jit instead of shipping zeros over the tunnel.
Weight densification happens on host (weights are tiny).
"""
import sys
sys.path.insert(0, '/opt/trn_rl_repo')
import hashlib
import numpy as np

B_TOTAL = 32768
N_CORES = 8
N_PER_CORE = B_TOTAL // N_CORES   # 4096
NT = 512                          # samples per supertile
NUM_MID = 6
N_CHUNKS = 1                      # execs per call (1: in-order cmd queue makes
                                  # any exec between puts stall the H2D stream)
N_XPARTS = 1                      # x dram tensors per core (1: upload is per
                                  # device instead, in device-major order)
NPC = N_PER_CORE // N_CHUNKS

_CACHE = {}


def _band(y):
    return [yi for yi in (y - 1, y, y + 1) if 0 <= yi <= 4]


def _densify_mid(w):
    """w [25,25,3,3] OIHW -> [13,125,125] blocks (lhsT: [q_in, q_out])."""
    blocks = np.zeros((13, 125, 125), np.float32)
    bi = 0
    for y_out in range(5):
        for y_in in _band(y_out):
            dy = y_in - y_out
            for x_in in range(5):
                for x_out in range(5):
                    dx = x_in - x_out
                    if abs(dx) <= 1:
                        blocks[bi, x_in*25:(x_in+1)*25, x_out*25:(x_out+1)*25] = \
                            w[:, :, dy+1, dx+1].T
            bi += 1
    return blocks


def _densify_conv0(w):
    """w [25,8,3,3] -> [200,625]: row c_in*25+y_in*5+x_in, col y_out*125+x_out*25+c_out."""
    W = np.zeros((200, 625), np.float32)
    for y_in in range(5):
        for x_in in range(5):
            for y_out in range(5):
                dy = y_in - y_out
                if abs(dy) > 1:
                    continue
                for x_out in range(5):
                    dx = x_in - x_out
                    if abs(dx) > 1:
                        continue
                    for c_in in range(8):
                        W[c_in*25 + y_in*5 + x_in,
                          y_out*125 + x_out*25:y_out*125 + x_out*25 + 25] = \
                            w[:, c_in, dy+1, dx+1]
    return W


def _densify_last(w):
    """w [6,25,3,3] -> [625,150]: row y_in*125+x_in*25+c_in, col wi*25+y_out*5+x_out."""
    W = np.zeros((625, 150), np.float32)
    for y_in in range(5):
        for x_in in range(5):
            for y_out in range(5):
                dy = y_in - y_out
                if abs(dy) > 1:
                    continue
                for x_out in range(5):
                    dx = x_in - x_out
                    if abs(dx) > 1:
                        continue
                    for c_in in range(25):
                        for wi in range(6):
                            W[y_in*125 + x_in*25 + c_in, wi*25 + y_out*5 + x_out] = \
                                w[wi, c_in, dy+1, dx+1]
    return W


def _build(npc):
    import concourse.bass as bass
    from concourse import bacc
    import concourse.tile as tile
    import concourse.mybir as mybir

    dt = mybir.dt
    AF = mybir.ActivationFunctionType
    ALU = mybir.AluOpType

    nc = bacc.Bacc("TRN2", target_bir_lowering=False, debug=False)
    n_st = npc // NT

    f32, f32r, f16 = dt.float32, dt.float32r, dt.float16
    npp = npc // N_XPARTS            # rows per x part
    x_ds = [nc.dram_tensor(f"x{p}", [npp, 200], f16, kind="ExternalInput").ap()
            for p in range(N_XPARTS)]
    y_d = nc.dram_tensor("y", [npc, 75], f16, kind="ExternalOutput").ap()
    w0a_d = nc.dram_tensor("w0a", [128, 625], f32, kind="ExternalInput").ap()
    w0b_d = nc.dram_tensor("w0b", [72, 625], f32, kind="ExternalInput").ap()
    wm_d = nc.dram_tensor("wm", [125, NUM_MID, 13, 125], f32, kind="ExternalInput").ap()
    wl_d = nc.dram_tensor("wl", [125, 5, 150], f32, kind="ExternalInput").ap()
    wp_d = nc.dram_tensor("wp", [75, 18], f32, kind="ExternalInput").ap()
    id_d = nc.dram_tensor("ident", [128, 128], f32, kind="ExternalInput").ap()
    id16_d = nc.dram_tensor("ident16", [128, 128], f16, kind="ExternalInput").ap()
    b0_d = nc.dram_tensor("b0q", [125, 1], f32, kind="ExternalInput").ap()
    bm_d = nc.dram_tensor("bmq", [125, NUM_MID], f32, kind="ExternalInput").ap()
    bl_d = nc.dram_tensor("blq", [75, 2], f32, kind="ExternalInput").ap()
    bp_d = nc.dram_tensor("bpq", [18, 1], f32, kind="ExternalInput").ap()

    with tile.TileContext(nc) as tc:
        with tc.tile_pool(name="wpool", bufs=1) as wpool, \
             tc.tile_pool(name="apool", bufs=3) as apool, \
             tc.tile_pool(name="npool", bufs=6) as npool, \
             tc.tile_pool(name="tpool", bufs=6) as tpool, \
             tc.tile_pool(name="pspool", bufs=8, space="PSUM") as pspool:

            w0a = wpool.tile([128, 625], f32r)
            w0b = wpool.tile([72, 625], f32r)
            wm = wpool.tile([125, NUM_MID, 13, 125], f32r)
            wl = wpool.tile([125, 5, 150], f32r)
            wp = wpool.tile([75, 18], f32r)
            ident = wpool.tile([128, 128], f32r)
            ident16 = wpool.tile([128, 128], f16)
            b0q = wpool.tile([125, 1], f32)
            bmq = wpool.tile([125, NUM_MID], f32)
            blq = wpool.tile([75, 2], f32)
            bpq = wpool.tile([18, 1], f32)
            nc.sync.dma_start(out=w0a, in_=w0a_d.bitcast(f32r))
            nc.sync.dma_start(out=w0b, in_=w0b_d.bitcast(f32r))
            nc.sync.dma_start(out=wm, in_=wm_d.bitcast(f32r))
            nc.sync.dma_start(out=wl, in_=wl_d.bitcast(f32r))
            nc.sync.dma_start(out=wp, in_=wp_d.bitcast(f32r))
            nc.sync.dma_start(out=ident, in_=id_d.bitcast(f32r))
            nc.sync.dma_start(out=ident16, in_=id16_d)
            nc.sync.dma_start(out=b0q, in_=b0_d)
            nc.sync.dma_start(out=bmq, in_=bm_d)
            nc.sync.dma_start(out=blq, in_=bl_d)
            nc.sync.dma_start(out=bpq, in_=bp_d)

            for s in range(n_st):
                base = s * NT
                # --- entry: DMA natural fp16 tiles, PE-transpose to feature-major
                xA = apool.tile([128, NT], f32r)
                xB = apool.tile([72, NT], f32r)
                for g in range(4):
                    nat = npool.tile([128, 200], f16, tag="nat")
                    r0 = base + g*128
                    nc.sync.dma_start(
                        out=nat, in_=x_ds[r0 // npp][r0 % npp:r0 % npp + 128, :])
                    psA = pspool.tile([128, 128], f16, tag="ps")
                    nc.tensor.transpose(psA, nat[:, 0:128], ident16)
                    nc.vector.tensor_copy(xA[:, g*128:(g+1)*128], psA)
                    psB = pspool.tile([72, 128], f16, tag="ps")
                    nc.tensor.transpose(psB, nat[:, 128:200], ident16)
                    nc.vector.tensor_copy(xB[:, g*128:(g+1)*128], psB)

                # --- conv0 (dense 200->625)
                h = apool.tile([125, 5, NT], f32r, tag="h")
                for y in range(5):
                    ps = pspool.tile([125, NT], f32, tag="ps")
                    nc.tensor.matmul(ps, w0a[:, y*125:(y+1)*125], xA,
                                     start=True, stop=False)
                    nc.tensor.matmul(ps, w0b[:, y*125:(y+1)*125], xB,
                                     start=False, stop=True)
                    if y >= 3:  # balance eviction load ACT vs DVE
                        nc.vector.tensor_scalar(h[:, y, :], ps, b0q, 0.0,
                                                op0=ALU.add, op1=ALU.max)
                    else:
                        nc.scalar.activation(h[:, y, :], ps, AF.Relu, bias=b0q)

                # --- 6 mid layers (row-banded 625->625)
                for l in range(NUM_MID):
                    hn = apool.tile([125, 5, NT], f32r, tag="h")
                    for y in range(5):
                        bnd = _band(y)
                        bi = sum(len(_band(yy)) for yy in range(y))
                        ps = pspool.tile([125, NT], f32, tag="ps")
                        for j, y_in in enumerate(bnd):
                            nc.tensor.matmul(ps, wm[:, l, bi+j, :], h[:, y_in, :],
                                             start=(j == 0), stop=(j == len(bnd)-1))
                        if y >= 3:
                            nc.vector.tensor_scalar(hn[:, y, :], ps,
                                                    bmq[:, l:l+1], 0.0,
                                                    op0=ALU.add, op1=ALU.max)
                        else:
                            nc.scalar.activation(hn[:, y, :], ps, AF.Relu,
                                                 bias=bmq[:, l:l+1])
                    h = hn

                # --- last layer (625->150, logits, w-major cols)
                hl = apool.tile([75, 2, NT], f32r)
                for m in range(2):
                    ps = pspool.tile([75, NT], f32, tag="ps")
                    for k in range(5):
                        nc.tensor.matmul(ps, wl[:, k, m*75:(m+1)*75], h[:, k, :],
                                         start=(k == 0), stop=(k == 4))
                    nc.scalar.activation(hl[:, m, :], ps, AF.Identity,
                                         bias=blq[:, m:m+1])

                # --- post conv (colors: 75->18)
                colors = apool.tile([18, NT], f32r)
                psc = pspool.tile([18, NT], f32, tag="ps")
                nc.tensor.matmul(psc, wp, xA[0:75, :], start=True, stop=True)
                nc.scalar.activation(colors, psc, AF.Identity, bias=bpq)

                # --- tail: per 128-group, sample-major softmax + color mix
                for g in range(4):
                    gs = slice(g*128, (g+1)*128)
                    # fp32r matmul ISA restriction: innermost free n_step must
                    # be even on moving operand and dst -> pad 75 to 76.
                    tE0 = pspool.tile([128, 76], f32r, tag="ps")
                    nc.tensor.transpose(tE0, hl[:, 0, gs], ident[0:75, 0:76])
                    tE1 = pspool.tile([128, 76], f32r, tag="ps")
                    nc.tensor.transpose(tE1, hl[:, 1, gs], ident[0:75, 0:76])
                    E = tpool.tile([128, 150], f32, tag="E")
                    nc.scalar.activation(E[:, 0:75], tE0[:, 0:75], AF.Exp)
                    nc.scalar.activation(E[:, 75:150], tE1[:, 0:75], AF.Exp)
                    tC = pspool.tile([128, 18], f32r, tag="ps")
                    nc.tensor.transpose(tC, colors[:, gs], ident[0:18, 0:18])
                    colT = tpool.tile([128, 18], f32, tag="colT")
                    nc.scalar.activation(colT, tC, AF.Copy)

                    S = tpool.tile([128, 25], f32, tag="S")
                    nc.vector.tensor_reduce(
                        out=S, in_=E.rearrange("p (w q) -> p q w", w=6),
                        axis=mybir.AxisListType.X, op=ALU.add)
                    R = tpool.tile([128, 25], f32, tag="R")
                    nc.vector.reciprocal(R, S)

                    U = tpool.tile([128, 3, 25], f32, tag="U")
                    for c in range(3):
                        nc.vector.tensor_scalar_mul(
                            U[:, c, :], E[:, 0:25], colT[:, c*6:c*6+1])
                        for w in range(1, 6):
                            nc.vector.scalar_tensor_tensor(
                                out=U[:, c, :], in0=E[:, w*25:(w+1)*25],
                                scalar=colT[:, c*6+w:c*6+w+1], in1=U[:, c, :],
                                op0=ALU.mult, op1=ALU.add)
                    F = tpool.tile([128, 3, 25], f16, tag="F")
                    nc.vector.tensor_tensor(
                        out=F, in0=U,
                        in1=R.unsqueeze(1).broadcast_to([128, 3, 25]),
                        op=ALU.mult)
                    nc.sync.dma_start(
                        out=y_d[base+g*128:base+(g+1)*128, :],
                        in_=F.rearrange("p a b -> p (a b)"))

    nc.compile()
    return nc


def _prep_weights(w0, b0, wmid, bmid, wlast, blast, wpost, bpost):
    W0 = _densify_conv0(np.asarray(w0, np.float32))
    wm = np.zeros((125, NUM_MID, 13, 125), np.float32)
    for l in range(NUM_MID):
        blocks = _densify_mid(np.asarray(wmid[l], np.float32))
        for bi in range(13):
            wm[:, l, bi, :] = blocks[bi]
    Wl = _densify_last(np.asarray(wlast, np.float32))
    wl = np.ascontiguousarray(
        np.transpose(Wl.reshape(5, 125, 150), (1, 0, 2)))
    wp = np.ascontiguousarray(
        np.asarray(wpost, np.float32).reshape(18, 75).T)
    b0q = np.tile(np.asarray(b0, np.float32), 5)[:, None]
    bmq = np.stack([np.tile(np.asarray(bmid[l], np.float32), 5)
                    for l in range(NUM_MID)], axis=1)
    blq = np.asarray(blast, np.float32).repeat(25).reshape(2, 75).T
    bpq = np.asarray(bpost, np.float32)[:, None]
    return {
        "w0a": np.ascontiguousarray(W0[0:128]),
        "w0b": np.ascontiguousarray(W0[128:200]),
        "wm": wm, "wl": wl, "wp": wp,
        "ident": np.eye(128, dtype=np.float32),
        "ident16": np.eye(128, dtype=np.float16),
        "b0q": np.ascontiguousarray(b0q), "bmq": np.ascontiguousarray(bmq),
        "blq": np.ascontiguousarray(blq), "bpq": bpq,
    }


def _get_exec(npc=NPC):
    """Build + compile the Bass program once per per-core batch size; cache
    a reusable jitted executable (the axon path re-traces jax.jit per call
    otherwise)."""
    if ("exec", npc, N_XPARTS) in _CACHE:
        return _CACHE[("exec", npc, N_XPARTS)]
    import jax
    import jax.numpy as jnp
    from jax.sharding import Mesh, PartitionSpec, NamedSharding
    from jax.experimental.shard_map import shard_map
    from concourse import mybir
    from concourse.bass2jax import (
        _bass_exec_p, install_neuronx_cc_hook, partition_id_tensor)

    install_neuronx_cc_hook()
    nc = _build(npc)

    partition_name = (nc.partition_id_tensor.name
                      if nc.partition_id_tensor else None)
    in_names, out_names, out_avals = [], [], []
    for alloc in nc.m.functions[0].allocations:
        if not isinstance(alloc, mybir.MemoryLocationSet):
            continue
        name = alloc.memorylocations[0].name
        if alloc.kind == "ExternalInput":
            if name != partition_name:
                in_names.append(name)
        elif alloc.kind == "ExternalOutput":
            out_names.append(name)
            out_avals.append(jax.core.ShapedArray(
                tuple(alloc.tensor_shape), mybir.dt.np(alloc.dtype)))
    n_params = len(in_names)
    in_names_full = in_names + out_names + (
        [partition_name] if partition_name else [])

    def _body(*args):
        operands = list(args)
        if partition_name is not None:
            operands.append(partition_id_tensor())
        outs = _bass_exec_p.bind(
            *operands, out_avals=tuple(out_avals),
            in_names=tuple(in_names_full), out_names=tuple(out_names),
            lowering_input_output_aliases=(), sim_require_finite=True,
            sim_require_nnan=True, nc=nc)
        return tuple(outs)

    devices = jax.devices()[:N_CORES]
    mesh = Mesh(np.asarray(devices), ("core",))
    S8 = NamedSharding(mesh, PartitionSpec("core"))
    n_outs = len(out_names)
    sharded = jax.jit(
        shard_map(_body, mesh=mesh,
                  in_specs=(PartitionSpec("core"),) * (n_params + n_outs),
                  out_specs=(PartitionSpec("core"),) * n_outs,
                  check_rep=False),
        donate_argnums=tuple(range(n_params, n_params + n_outs)),
        keep_unused=True)
    zeros_jit = jax.jit(
        lambda: jnp.zeros((N_CORES * npc, 75), jnp.float16),
        out_shardings=S8)

    ex = {"nc": nc, "sharded": sharded, "zeros_jit": zeros_jit,
          "in_names": in_names, "sharding": S8, "jax": jax,
          "ybufs": [], "npc": npc}
    _CACHE[("exec", npc, N_XPARTS)] = ex
    return ex


FETCH_THREADS = 8     # <=1 : serial np.asarray on the global sharded array
_POOL = None


def _to_f16(src):
    """f32 ndarray (any strides) -> contiguous f16 ndarray. torch's vectorized
    cast is ~3x faster than numpy's; fall back to numpy if unavailable."""
    cvt = _CACHE.get("torch_cvt")
    if cvt is None:
        try:
            import torch
            cvt = lambda a: torch.from_numpy(a).to(torch.float16).numpy()
            cvt(np.zeros((2, 2), np.float32))
        except Exception:
            cvt = lambda a: np.ascontiguousarray(a, dtype=np.float16)
        _CACHE["torch_cvt"] = cvt
    return cvt(src)


def _pool():
    global _POOL
    if _POOL is None:
        from concurrent.futures import ThreadPoolExecutor
        _POOL = ThreadPoolExecutor(max_workers=8)
    return _POOL


def _fetch_into(y, out_view):
    """Device sharded [R,75] f16 -> out_view [N_CORES, R//N_CORES, 75] f32."""
    if FETCH_THREADS <= 1:
        out_view[...] = np.asarray(y).reshape(out_view.shape)
        return
    shards = y.addressable_shards
    def fx(i):
        s = shards[i]
        out_view[i] = np.asarray(s.data).reshape(out_view.shape[1:])
    list(_pool().map(fx, range(len(shards))))


def kernel(input, w0, b0, wmid, bmid, wlast, blast, wpost, bpost, _trace=False):
    npc = N_PER_CORE // N_CHUNKS
    ex = _get_exec(npc)
    jax = ex["jax"]

    # cache densified+device-resident weights keyed by raw weight bytes
    hsh = hashlib.blake2b(digest_size=16)
    for a in (w0, b0, wmid, bmid, wlast, blast, wpost, bpost):
        hsh.update(np.ascontiguousarray(a).tobytes())
    wkey = hsh.hexdigest()
    if _CACHE.get("wkey") != wkey:
        wmap = _prep_weights(w0, b0, wmid, bmid, wlast, blast, wpost, bpost)
        dev_w = {}
        for name, arr in wmap.items():
            rep = np.concatenate([arr] * N_CORES, axis=0)
            dev_w[name] = jax.device_put(rep, ex["sharding"])
        _CACHE["dev_w"] = dev_w
        _CACHE["wkey"] = wkey

    # Issue order matters on the axon tunnel (in-order command queue, async
    # pulls): ALL uploads first, then all execs, then fetches. Each exec is
    # gated only on its own chunk's transfer, so chunk k's exec + y fetch
    # (slow D2H direction, ~35MB/s) overlap the upload of chunks k+1.. An
    # exec issued BETWEEN puts would stall the upload stream (~75ms each).
    # The donated output buffers are the previous call's device-resident
    # outputs (the kernel writes every element, so contents don't matter)
    # -- no per-call zero-fill exec.
    npp = npc // N_XPARTS
    xv = np.asarray(input).reshape(N_CORES, N_CHUNKS, N_XPARTS, npp, 200)
    ybufs = ex["ybufs"]
    while len(ybufs) < N_CHUNKS:
        ybufs.append(ex["zeros_jit"]())
    ex["ybufs"] = []   # consumed by donation below; refill on success
    warg = [_CACHE["dev_w"].get(n) for n in ex["in_names"]]
    xpos = [ex["in_names"].index(f"x{p}") for p in range(N_XPARTS)]
    xds = []
    for k in range(N_CHUNKS):
        for p in range(N_XPARTS):
            x16 = _to_f16(xv[:, k, p])
            xds.append(jax.device_put(
                x16.reshape(N_CORES * npp, 200), ex["sharding"]))
    ys = []
    for k in range(N_CHUNKS):
        for p in range(N_XPARTS):
            warg[xpos[p]] = xds[k * N_XPARTS + p]
        (y,) = ex["sharded"](*warg, ybufs[k])
        ys.append(y)
    out = np.empty((N_CORES, N_CHUNKS, N_XPARTS * npp, 75), np.float32)
    for k in range(N_CHUNKS):
        _fetch_into(ys[k], out[:, k])
    ex["ybufs"] = ys   # donate these next call
    return out.reshape(B_TOTAL, 75).reshape(B_TOTAL, 3, 5, 5)


# revision 28
# speedup vs baseline: 1.0537x; 1.0186x over previous
"""KPCNN (kernel-predicting CNN) Trainium2 Bass kernel.

Strategy (hardcoded for B=32768, 8 cores, pure data parallel, 4096 samples/core):
 - All convs on 5x5 spatial are reformulated as dense matmuls over flattened
   (pixel, channel) feature vectors, row-banded by output image row so each
   125-wide output chunk contracts only the 2-3 input row chunks in its 3x3
   band (13 [125x125] blocks per 25->25 layer instead of 25).
 - Activations are feature-major [feat, batch] in SBUF, fp32r (TF32-like,
   full PE rate at N>=256), PSUM accumulate fp32.
 - Entry: PE-transpose of naturally-DMA'd [128 samples, 200 feat] tiles.
 - Tail (softmax over 6 predicted weights + per-pixel color mix) runs
   sample-major after PE-transposing back, on DVE/ACT.
 - Wall-clock is dominated by the axon tunnel (~100MB/s H2D, slower D2H,
   ~50-80ms per serialized command round trip), so the host path minimizes
   bytes and tunnel operations: fp16 input/output (13.1MB up, 4.9MB down),
   densified weights + the jitted executable cached on device across calls
   (keyed by weight-byte hash), a single exec per call issued strictly after
   the upload (the in-order command queue stalls transfers behind an
   interleaved exec), donated output buffers recycled from the previous
   call's device-resident outputs (no per-call zero-fill exec), and
   concurrent per-shard output pulls.
Weight densification happens on host (weights are tiny).
"""
import sys
sys.path.insert(0, '/opt/trn_rl_repo')
import hashlib
import numpy as np

B_TOTAL = 32768
N_CORES = 8
N_PER_CORE = B_TOTAL // N_CORES   # 4096
NT = 512                          # samples per supertile
NUM_MID = 6
N_CHUNKS = 1                      # execs per call (1: in-order cmd queue makes
                                  # any exec between puts stall the H2D stream)
N_XPARTS = 1                      # x dram tensors per core (1: upload is per
                                  # device instead, in device-major order)
NPC = N_PER_CORE // N_CHUNKS

_CACHE = {}


def _band(y):
    return [yi for yi in (y - 1, y, y + 1) if 0 <= yi <= 4]


def _densify_mid(w):
    """w [25,25,3,3] OIHW -> [13,125,125] blocks (lhsT: [q_in, q_out])."""
    blocks = np.zeros((13, 125, 125), np.float32)
    bi = 0
    for y_out in range(5):
        for y_in in _band(y_out):
            dy = y_in - y_out
            for x_in in range(5):
                for x_out in range(5):
                    dx = x_in - x_out
                    if abs(dx) <= 1:
                        blocks[bi, x_in*25:(x_in+1)*25, x_out*25:(x_out+1)*25] = \
                            w[:, :, dy+1, dx+1].T
            bi += 1
    return blocks


def _densify_conv0(w):
    """w [25,8,3,3] -> [200,625]: row c_in*25+y_in*5+x_in, col y_out*125+x_out*25+c_out."""
    W = np.zeros((200, 625), np.float32)
    for y_in in range(5):
        for x_in in range(5):
            for y_out in range(5):
                dy = y_in - y_out
                if abs(dy) > 1:
                    continue
                for x_out in range(5):
                    dx = x_in - x_out
                    if abs(dx) > 1:
                        continue
                    for c_in in range(8):
                        W[c_in*25 + y_in*5 + x_in,
                          y_out*125 + x_out*25:y_out*125 + x_out*25 + 25] = \
                            w[:, c_in, dy+1, dx+1]
    return W


def _densify_last(w):
    """w [6,25,3,3] -> [625,150]: row y_in*125+x_in*25+c_in, col wi*25+y_out*5+x_out."""
    W = np.zeros((625, 150), np.float32)
    for y_in in range(5):
        for x_in in range(5):
            for y_out in range(5):
                dy = y_in - y_out
                if abs(dy) > 1:
                    continue
                for x_out in range(5):
                    dx = x_in - x_out
                    if abs(dx) > 1:
                        continue
                    for c_in in range(25):
                        for wi in range(6):
                            W[y_in*125 + x_in*25 + c_in, wi*25 + y_out*5 + x_out] = \
                                w[wi, c_in, dy+1, dx+1]
    return W


def _build(npc):
    import concourse.bass as bass
    from concourse import bacc
    import concourse.tile as tile
    import concourse.mybir as mybir

    dt = mybir.dt
    AF = mybir.ActivationFunctionType
    ALU = mybir.AluOpType

    nc = bacc.Bacc("TRN2", target_bir_lowering=False, debug=False)
    n_st = npc // NT

    f32, f32r, f16 = dt.float32, dt.float32r, dt.float16
    npp = npc // N_XPARTS            # rows per x part
    x_ds = [nc.dram_tensor(f"x{p}", [npp, 200], f16, kind="ExternalInput").ap()
            for p in range(N_XPARTS)]
    y_d = nc.dram_tensor("y", [npc, 75], f16, kind="ExternalOutput").ap()
    w0a_d = nc.dram_tensor("w0a", [128, 625], f32, kind="ExternalInput").ap()
    w0b_d = nc.dram_tensor("w0b", [72, 625], f32, kind="ExternalInput").ap()
    wm_d = nc.dram_tensor("wm", [125, NUM_MID, 13, 125], f32, kind="ExternalInput").ap()
    wl_d = nc.dram_tensor("wl", [125, 5, 150], f32, kind="ExternalInput").ap()
    wp_d = nc.dram_tensor("wp", [75, 18], f32, kind="ExternalInput").ap()
    id_d = nc.dram_tensor("ident", [128, 128], f32, kind="ExternalInput").ap()
    id16_d = nc.dram_tensor("ident16", [128, 128], f16, kind="ExternalInput").ap()
    b0_d = nc.dram_tensor("b0q", [125, 1], f32, kind="ExternalInput").ap()
    bm_d = nc.dram_tensor("bmq", [125, NUM_MID], f32, kind="ExternalInput").ap()
    bl_d = nc.dram_tensor("blq", [75, 2], f32, kind="ExternalInput").ap()
    bp_d = nc.dram_tensor("bpq", [18, 1], f32, kind="ExternalInput").ap()

    with tile.TileContext(nc) as tc:
        with tc.tile_pool(name="wpool", bufs=1) as wpool, \
             tc.tile_pool(name="apool", bufs=3) as apool, \
             tc.tile_pool(name="npool", bufs=6) as npool, \
             tc.tile_pool(name="tpool", bufs=6) as tpool, \
             tc.tile_pool(name="pspool", bufs=8, space="PSUM") as pspool:

            w0a = wpool.tile([128, 625], f32r)
            w0b = wpool.tile([72, 625], f32r)
            wm = wpool.tile([125, NUM_MID, 13, 125], f32r)
            wl = wpool.tile([125, 5, 150], f32r)
            wp = wpool.tile([75, 18], f32r)
            ident = wpool.tile([128, 128], f32r)
            ident16 = wpool.tile([128, 128], f16)
            b0q = wpool.tile([125, 1], f32)
            bmq = wpool.tile([125, NUM_MID], f32)
            blq = wpool.tile([75, 2], f32)
            bpq = wpool.tile([18, 1], f32)
            nc.sync.dma_start(out=w0a, in_=w0a_d.bitcast(f32r))
            nc.sync.dma_start(out=w0b, in_=w0b_d.bitcast(f32r))
            nc.sync.dma_start(out=wm, in_=wm_d.bitcast(f32r))
            nc.sync.dma_start(out=wl, in_=wl_d.bitcast(f32r))
            nc.sync.dma_start(out=wp, in_=wp_d.bitcast(f32r))
            nc.sync.dma_start(out=ident, in_=id_d.bitcast(f32r))
            nc.sync.dma_start(out=ident16, in_=id16_d)
            nc.sync.dma_start(out=b0q, in_=b0_d)
            nc.sync.dma_start(out=bmq, in_=bm_d)
            nc.sync.dma_start(out=blq, in_=bl_d)
            nc.sync.dma_start(out=bpq, in_=bp_d)

            for s in range(n_st):
                base = s * NT
                # --- entry: DMA natural fp16 tiles, PE-transpose to feature-major
                xA = apool.tile([128, NT], f32r)
                xB = apool.tile([72, NT], f32r)
                for g in range(4):
                    nat = npool.tile([128, 200], f16, tag="nat")
                    r0 = base + g*128
                    nc.sync.dma_start(
                        out=nat, in_=x_ds[r0 // npp][r0 % npp:r0 % npp + 128, :])
                    psA = pspool.tile([128, 128], f16, tag="ps")
                    nc.tensor.transpose(psA, nat[:, 0:128], ident16)
                    nc.vector.tensor_copy(xA[:, g*128:(g+1)*128], psA)
                    psB = pspool.tile([72, 128], f16, tag="ps")
                    nc.tensor.transpose(psB, nat[:, 128:200], ident16)
                    nc.vector.tensor_copy(xB[:, g*128:(g+1)*128], psB)

                # --- conv0 (dense 200->625)
                h = apool.tile([125, 5, NT], f32r, tag="h")
                for y in range(5):
                    ps = pspool.tile([125, NT], f32, tag="ps")
                    nc.tensor.matmul(ps, w0a[:, y*125:(y+1)*125], xA,
                                     start=True, stop=False)
                    nc.tensor.matmul(ps, w0b[:, y*125:(y+1)*125], xB,
                                     start=False, stop=True)
                    if y >= 3:  # balance eviction load ACT vs DVE
                        nc.vector.tensor_scalar(h[:, y, :], ps, b0q, 0.0,
                                                op0=ALU.add, op1=ALU.max)
                    else:
                        nc.scalar.activation(h[:, y, :], ps, AF.Relu, bias=b0q)

                # --- 6 mid layers (row-banded 625->625)
                for l in range(NUM_MID):
                    hn = apool.tile([125, 5, NT], f32r, tag="h")
                    for y in range(5):
                        bnd = _band(y)
                        bi = sum(len(_band(yy)) for yy in range(y))
                        ps = pspool.tile([125, NT], f32, tag="ps")
                        for j, y_in in enumerate(bnd):
                            nc.tensor.matmul(ps, wm[:, l, bi+j, :], h[:, y_in, :],
                                             start=(j == 0), stop=(j == len(bnd)-1))
                        if y >= 3:
                            nc.vector.tensor_scalar(hn[:, y, :], ps,
                                                    bmq[:, l:l+1], 0.0,
                                                    op0=ALU.add, op1=ALU.max)
                        else:
                            nc.scalar.activation(hn[:, y, :], ps, AF.Relu,
                                                 bias=bmq[:, l:l+1])
                    h = hn

                # --- last layer (625->150, logits, w-major cols)
                hl = apool.tile([75, 2, NT], f32r)
                for m in range(2):
                    ps = pspool.tile([75, NT], f32, tag="ps")
                    for k in range(5):
                        nc.tensor.matmul(ps, wl[:, k, m*75:(m+1)*75], h[:, k, :],
                                         start=(k == 0), stop=(k == 4))
                    nc.scalar.activation(hl[:, m, :], ps, AF.Identity,
                                         bias=blq[:, m:m+1])

                # --- post conv (colors: 75->18)
                colors = apool.tile([18, NT], f32r)
                psc = pspool.tile([18, NT], f32, tag="ps")
                nc.tensor.matmul(psc, wp, xA[0:75, :], start=True, stop=True)
                nc.scalar.activation(colors, psc, AF.Identity, bias=bpq)

                # --- tail: per 128-group, sample-major softmax + color mix
                for g in range(4):
                    gs = slice(g*128, (g+1)*128)
                    # fp32r matmul ISA restriction: innermost free n_step must
                    # be even on moving operand and dst -> pad 75 to 76.
                    tE0 = pspool.tile([128, 76], f32r, tag="ps")
                    nc.tensor.transpose(tE0, hl[:, 0, gs], ident[0:75, 0:76])
                    tE1 = pspool.tile([128, 76], f32r, tag="ps")
                    nc.tensor.transpose(tE1, hl[:, 1, gs], ident[0:75, 0:76])
                    E = tpool.tile([128, 150], f32, tag="E")
                    nc.scalar.activation(E[:, 0:75], tE0[:, 0:75], AF.Exp)
                    nc.scalar.activation(E[:, 75:150], tE1[:, 0:75], AF.Exp)
                    tC = pspool.tile([128, 18], f32r, tag="ps")
                    nc.tensor.transpose(tC, colors[:, gs], ident[0:18, 0:18])
                    colT = tpool.tile([128, 18], f32, tag="colT")
                    nc.scalar.activation(colT, tC, AF.Copy)

                    S = tpool.tile([128, 25], f32, tag="S")
                    nc.vector.tensor_reduce(
                        out=S, in_=E.rearrange("p (w q) -> p q w", w=6),
                        axis=mybir.AxisListType.X, op=ALU.add)
                    R = tpool.tile([128, 25], f32, tag="R")
                    nc.vector.reciprocal(R, S)

                    U = tpool.tile([128, 3, 25], f32, tag="U")
                    for c in range(3):
                        nc.vector.tensor_scalar_mul(
                            U[:, c, :], E[:, 0:25], colT[:, c*6:c*6+1])
                        for w in range(1, 6):
                            nc.vector.scalar_tensor_tensor(
                                out=U[:, c, :], in0=E[:, w*25:(w+1)*25],
                                scalar=colT[:, c*6+w:c*6+w+1], in1=U[:, c, :],
                                op0=ALU.mult, op1=ALU.add)
                    F = tpool.tile([128, 3, 25], f16, tag="F")
                    nc.vector.tensor_tensor(
                        out=F, in0=U,
                        in1=R.unsqueeze(1).broadcast_to([128, 3, 25]),
                        op=ALU.mult)
                    nc.sync.dma_start(
                        out=y_d[base+g*128:base+(g+1)*128, :],
                        in_=F.rearrange("p a b -> p (a b)"))

    nc.compile()
    return nc


def _prep_weights(w0, b0, wmid, bmid, wlast, blast, wpost, bpost):
    W0 = _densify_conv0(np.asarray(w0, np.float32))
    wm = np.zeros((125, NUM_MID, 13, 125), np.float32)
    for l in range(NUM_MID):
        blocks = _densify_mid(np.asarray(wmid[l], np.float32))
        for bi in range(13):
            wm[:, l, bi, :] = blocks[bi]
    Wl = _densify_last(np.asarray(wlast, np.float32))
    wl = np.ascontiguousarray(
        np.transpose(Wl.reshape(5, 125, 150), (1, 0, 2)))
    wp = np.ascontiguousarray(
        np.asarray(wpost, np.float32).reshape(18, 75).T)
    b0q = np.tile(np.asarray(b0, np.float32), 5)[:, None]
    bmq = np.stack([np.tile(np.asarray(bmid[l], np.float32), 5)
                    for l in range(NUM_MID)], axis=1)
    blq = np.asarray(blast, np.float32).repeat(25).reshape(2, 75).T
    bpq = np.asarray(bpost, np.float32)[:, None]
    return {
        "w0a": np.ascontiguousarray(W0[0:128]),
        "w0b": np.ascontiguousarray(W0[128:200]),
        "wm": wm, "wl": wl, "wp": wp,
        "ident": np.eye(128, dtype=np.float32),
        "ident16": np.eye(128, dtype=np.float16),
        "b0q": np.ascontiguousarray(b0q), "bmq": np.ascontiguousarray(bmq),
        "blq": np.ascontiguousarray(blq), "bpq": bpq,
    }


def _get_exec(npc=NPC):
    """Build + compile the Bass program once per per-core batch size; cache
    a reusable jitted executable (the axon path re-traces jax.jit per call
    otherwise)."""
    if ("exec", npc, N_XPARTS) in _CACHE:
        return _CACHE[("exec", npc, N_XPARTS)]
    import jax
    import jax.numpy as jnp
    from jax.sharding import Mesh, PartitionSpec, NamedSharding
    from jax.experimental.shard_map import shard_map
    from concourse import mybir
    from concourse.bass2jax import (
        _bass_exec_p, install_neuronx_cc_hook, partition_id_tensor)

    install_neuronx_cc_hook()
    nc = _build(npc)

    partition_name = (nc.partition_id_tensor.name
                      if nc.partition_id_tensor else None)
    in_names, out_names, out_avals = [], [], []
    for alloc in nc.m.functions[0].allocations:
        if not isinstance(alloc, mybir.MemoryLocationSet):
            continue
        name = alloc.memorylocations[0].name
        if alloc.kind == "ExternalInput":
            if name != partition_name:
                in_names.append(name)
        elif alloc.kind == "ExternalOutput":
            out_names.append(name)
            out_avals.append(jax.core.ShapedArray(
                tuple(alloc.tensor_shape), mybir.dt.np(alloc.dtype)))
    n_params = len(in_names)
    in_names_full = in_names + out_names + (
        [partition_name] if partition_name else [])

    def _body(*args):
        operands = list(args)
        if partition_name is not None:
            operands.append(partition_id_tensor())
        outs = _bass_exec_p.bind(
            *operands, out_avals=tuple(out_avals),
            in_names=tuple(in_names_full), out_names=tuple(out_names),
            lowering_input_output_aliases=(), sim_require_finite=True,
            sim_require_nnan=True, nc=nc)
        return tuple(outs)

    devices = jax.devices()[:N_CORES]
    mesh = Mesh(np.asarray(devices), ("core",))
    S8 = NamedSharding(mesh, PartitionSpec("core"))
    n_outs = len(out_names)
    sharded = jax.jit(
        shard_map(_body, mesh=mesh,
                  in_specs=(PartitionSpec("core"),) * (n_params + n_outs),
                  out_specs=(PartitionSpec("core"),) * n_outs,
                  check_rep=False),
        donate_argnums=tuple(range(n_params, n_params + n_outs)),
        keep_unused=True)
    zeros_jit = jax.jit(
        lambda: jnp.zeros((N_CORES * npc, 75), jnp.float16),
        out_shardings=S8)

    ex = {"nc": nc, "sharded": sharded, "zeros_jit": zeros_jit,
          "in_names": in_names, "sharding": S8, "jax": jax,
          "ybufs": [], "npc": npc}
    _CACHE[("exec", npc, N_XPARTS)] = ex
    return ex


FETCH_THREADS = 8     # <=1 : serial np.asarray on the global sharded array
_POOL = None


def _to_f16(src):
    """f32 ndarray (any strides) -> contiguous f16 ndarray. torch's vectorized
    cast is ~3x faster than numpy's; fall back to numpy if unavailable."""
    cvt = _CACHE.get("torch_cvt")
    if cvt is None:
        try:
            import warnings
            import torch
            warnings.filterwarnings(
                "ignore", message=".*is not writable.*", module="torch")
            warnings.filterwarnings(
                "ignore", message=".*is not writable.*")
            cvt = lambda a: torch.from_numpy(a).to(torch.float16).numpy()
            cvt(np.zeros((2, 2), np.float32))
        except Exception:
            cvt = lambda a: np.ascontiguousarray(a, dtype=np.float16)
        _CACHE["torch_cvt"] = cvt
    return cvt(src)


def _pool():
    global _POOL
    if _POOL is None:
        from concurrent.futures import ThreadPoolExecutor
        _POOL = ThreadPoolExecutor(max_workers=8)
    return _POOL


def _fetch_into(y, out_view):
    """Device sharded [R,75] f16 -> out_view [N_CORES, R//N_CORES, 75] f32."""
    if FETCH_THREADS <= 1:
        out_view[...] = np.asarray(y).reshape(out_view.shape)
        return
    shards = y.addressable_shards
    def fx(i):
        s = shards[i]
        out_view[i] = np.asarray(s.data).reshape(out_view.shape[1:])
    list(_pool().map(fx, range(len(shards))))


def kernel(input, w0, b0, wmid, bmid, wlast, blast, wpost, bpost, _trace=False):
    npc = N_PER_CORE // N_CHUNKS
    ex = _get_exec(npc)
    jax = ex["jax"]

    # cache densified+device-resident weights keyed by raw weight bytes
    hsh = hashlib.blake2b(digest_size=16)
    for a in (w0, b0, wmid, bmid, wlast, blast, wpost, bpost):
        hsh.update(np.ascontiguousarray(a).tobytes())
    wkey = hsh.hexdigest()
    if _CACHE.get("wkey") != wkey:
        wmap = _prep_weights(w0, b0, wmid, bmid, wlast, blast, wpost, bpost)
        dev_w = {}
        for name, arr in wmap.items():
            rep = np.concatenate([arr] * N_CORES, axis=0)
            dev_w[name] = jax.device_put(rep, ex["sharding"])
        _CACHE["dev_w"] = dev_w
        _CACHE["wkey"] = wkey

    # Issue order matters on the axon tunnel (in-order command queue, async
    # pulls): ALL uploads first, then all execs, then fetches. Each exec is
    # gated only on its own chunk's transfer, so chunk k's exec + y fetch
    # (slow D2H direction, ~35MB/s) overlap the upload of chunks k+1.. An
    # exec issued BETWEEN puts would stall the upload stream (~75ms each).
    # The donated output buffers are the previous call's device-resident
    # outputs (the kernel writes every element, so contents don't matter)
    # -- no per-call zero-fill exec.
    npp = npc // N_XPARTS
    xv = np.asarray(input).reshape(N_CORES, N_CHUNKS, N_XPARTS, npp, 200)
    ybufs = ex["ybufs"]
    while len(ybufs) < N_CHUNKS:
        ybufs.append(ex["zeros_jit"]())
    ex["ybufs"] = []   # consumed by donation below; refill on success
    warg = [_CACHE["dev_w"].get(n) for n in ex["in_names"]]
    xpos = [ex["in_names"].index(f"x{p}") for p in range(N_XPARTS)]
    xds = []
    for k in range(N_CHUNKS):
        for p in range(N_XPARTS):
            x16 = _to_f16(xv[:, k, p])
            xds.append(jax.device_put(
                x16.reshape(N_CORES * npp, 200), ex["sharding"]))
    ys = []
    for k in range(N_CHUNKS):
        for p in range(N_XPARTS):
            warg[xpos[p]] = xds[k * N_XPARTS + p]
        (y,) = ex["sharded"](*warg, ybufs[k])
        ys.append(y)
    out = np.empty((N_CORES, N_CHUNKS, N_XPARTS * npp, 75), np.float32)
    for k in range(N_CHUNKS):
        _fetch_into(ys[k], out[:, k])
    ex["ybufs"] = ys   # donate these next call
    return out.reshape(B_TOTAL, 75).reshape(B_TOTAL, 3, 5, 5)
